# revision 30
# baseline (speedup 1.0000x reference)
"""AdaProj kernel for 8 TRN2 NeuronCores.

Math reduction (validated vs reference to ~4e-6 max rel err in f32):
  out[b,c] = rnx_b * num / sqrt(den)
  num = sum_s (rnw_s L_s)^2
  den = num + sum_{s<s'} g2m_ss' * (m_s * m_s'),  m_s = rnw_s * L_s
  g2m = 2*Graw_ss'*rnw_s*rnw_s'  (per-class scalars)
  L_s[c,b] = W[c,s,:] . x[b,:]  (raw matmul), rnw = 1/||W_cs||, rnx = 1/||x_b||
This removes the [B,C,D] intermediate of the reference entirely.

Sharding: W split over classes C (125/core); x replicated. No collectives —
host concatenates the per-core [125, 256] outputs.
"""

import numpy as np
import ml_dtypes

import concourse.bacc as bacc
import concourse.bass as bass
import concourse.mybir as mybir
import concourse.tile as tile
from concourse.bass_utils import run_bass_kernel_spmd

B, C, S, D = 256, 1000, 4, 512
NCORES = 8
CS = C // NCORES  # 125 classes per core
R = CS * S        # 500 W rows per core
KP = D // 128     # 4 contraction chunks
PAIRS = [(0, 1), (0, 2), (0, 3), (1, 2), (1, 3), (2, 3)]

F32 = mybir.dt.float32
BF16 = mybir.dt.bfloat16
FP16 = mybir.dt.float16
AF = mybir.ActivationFunctionType
OP = mybir.AluOpType

_CACHED = {}


def _emit_body(nc, pool, psum, xT_d, wT_d, wcm_d, out_d, it, TIN, parts="all"):
    p = f"i{it}_"

    def st(shape, dtype, name, space_pool=None):
        sp = space_pool if space_pool is not None else pool
        return sp.tile(shape, dtype, tag=p + name, name=p + name)

    # ---------- activation-table warmup (sqrt_and_others covers all) ----
    warm = st([1, 1], F32, "warm")
    nc.vector.memset(warm[:], 1.0)
    warm3 = st([1, 1], F32, "warm3")
    nc.scalar.activation(warm3[:], warm[:], AF.Abs_reciprocal_sqrt)

    # ---------- inputs (separate tiles so tile-granular deps don't chain) --
    xt = st([128, KP, B], TIN, "xt")
    wt = st([128, KP, R], mybir.dt.float8e3, "wt")
    wcmA = st([CS, 2, D], TIN, "wcmA")  # s = 0,1
    wcmB = st([CS, 2, D], TIN, "wcmB")  # s = 2,3
    nc.sync.dma_start(wcmA[:, :, :], wcm_d[:, 0:2, :])
    nc.sync.dma_start(wt[:, :, :], wT_d[:, :, :])
    nc.sync.dma_start(
        xt[:, :, :], xT_d[:, :].rearrange("(k p) b -> p k b", p=128)
    )
    nc.sync.dma_start(wcmB[:, :, :], wcm_d[:, 2:4, :])

    def wslice(s):
        return wcmA[:, s, :] if s < 2 else wcmB[:, s - 2, :]

    # ---------- W norms, first half (s=0,1) ----------
    s2a = st([CS, 2], F32, "s2a")
    s2b = st([CS, 2], F32, "s2b")
    sq_scr = st([CS, S, D], F32, "sq_scr")
    rnwa = st([CS, 2], F32, "rnwa")
    rnwb = st([CS, 2], F32, "rnwb")
    for s in range(2):
        nc.scalar.activation(
            sq_scr[:, s, :], wcmA[:, s, :], AF.Square,
            accum_out=s2a[:, s:s + 1],
        )
    nc.scalar.activation(rnwa[:], s2a[:], AF.Abs_reciprocal_sqrt)

    def rnw_col(s):
        return rnwa[:, s:s + 1] if s < 2 else rnwb[:, s - 2:s - 1]

    # ---------- main matmuls (s-major so L_s completes incrementally) ----
    Lp = [st([CS, B], F32, f"L{s}", psum) for s in range(S)]
    last_L_mm = None
    for s in range(S):
        for k in range(KP):
            last_L_mm = nc.tensor.matmul(
                Lp[s][:],
                wt[:, k, s * CS:(s + 1) * CS],
                xt[:, k, :],
                start=(k == 0), stop=(k == KP - 1),
            )

    # ---------- W norms, second half (s=2,3) ----------
    for s in range(2):
        nc.scalar.activation(
            sq_scr[:, 2 + s, :], wcmB[:, s, :], AF.Square,
            accum_out=s2b[:, s:s + 1],
        )
    nc.scalar.activation(rnwb[:], s2b[:], AF.Abs_reciprocal_sqrt)

    # ---------- m_s = rnw_s * L_s on ScalarE ----------
    m = [st([CS, B], TIN, f"m{s}") for s in range(S)]
    m_insts = []
    for s in range(S):
        m_insts.append(nc.scalar.mul(m[s][:], Lp[s][:], rnw_col(s)))

    # ---------- Gram cross products (pair (0,1) can run earliest) --------
    prod1 = st([CS, 3, D], TIN, "prod1")  # (0,1),(1,2),(2,3)
    nc.vector.tensor_tensor(prod1[:, 0, :], wcmA[:, 0, :], wcmA[:, 1, :], OP.mult)
    nc.vector.tensor_tensor(prod1[:, 1, :], wcmA[:, 1, :], wcmB[:, 0, :], OP.mult)
    nc.vector.tensor_tensor(prod1[:, 2, :], wcmB[:, 0, :], wcmB[:, 1, :], OP.mult)
    prod2 = st([CS, 2, D], TIN, "prod2")  # (0,2),(1,3)
    nc.vector.tensor_tensor(prod2[:], wcmA[:, :, :], wcmB[:, :, :], OP.mult)
    prod3 = st([CS, 1, D], TIN, "prod3")  # (0,3)
    nc.gpsimd.tensor_tensor(prod3[:, 0, :], wcmA[:, 0, :], wcmB[:, 1, :], OP.mult)
    xsq = st([128, KP, B], TIN, "xsq")
    nc.gpsimd.tensor_tensor(xsq[:], xt[:], xt[:], OP.mult)

    gr1 = st([CS, 3], F32, "gr1")
    gr2a_t = st([CS, 1], F32, "gr2a_t")
    gr2b_t = st([CS, 1], F32, "gr2b_t")
    gr3 = st([CS, 1], F32, "gr3")
    red_scr = st([CS, 3, D], F32, "red_scr")
    nc.vector.tensor_reduce(gr1[:], prod1[:], mybir.AxisListType.X, OP.add)
    for j, grt in enumerate([gr2a_t, gr2b_t]):
        cp = nc.scalar.activation(
            red_scr[:, j, :], prod2[:, j, :], AF.Copy,
            accum_out=grt[:],
        )
        if j == 0:
            bass._add_dep_helper(
                cp.ins, m_insts[-1].ins, sync=False,
                reason="m copies go first on ScalarE",
            )
    nc.scalar.activation(
        red_scr[:, 2, :], prod3[:, 0, :], AF.Copy,
        accum_out=gr3[:, 0:1],
    )

    # ---------- epilogue: q/num path (V-queue ordered by readiness) ------
    q = [st([CS, B], TIN, f"q{s}") for s in range(S)]
    n01 = st([CS, B], TIN, "n01")
    n23 = st([CS, B], TIN, "n23")
    num = st([CS, B], TIN, "num")
    ps = [st([CS, B], TIN, f"p{i}") for i in range(6)]
    # work that only needs m0/m1 first
    nc.vector.tensor_tensor(q[0][:], m[0][:], m[0][:], OP.mult)
    nc.gpsimd.tensor_tensor(q[1][:], m[1][:], m[1][:], OP.mult)
    nc.vector.tensor_tensor(ps[0][:], m[0][:], m[1][:], OP.mult)
    nc.vector.tensor_tensor(n01[:], q[0][:], q[1][:], OP.add)
    # then m2-dependent, then m3-dependent
    nc.gpsimd.tensor_tensor(q[2][:], m[2][:], m[2][:], OP.mult)
    nc.vector.tensor_tensor(ps[1][:], m[0][:], m[2][:], OP.mult)
    nc.gpsimd.tensor_tensor(ps[3][:], m[1][:], m[2][:], OP.mult)
    nc.vector.tensor_tensor(q[3][:], m[3][:], m[3][:], OP.mult)
    nc.vector.tensor_tensor(n23[:], q[2][:], q[3][:], OP.add)
    nc.vector.tensor_tensor(num[:], n01[:], n23[:], OP.add)
    nc.gpsimd.tensor_tensor(ps[2][:], m[0][:], m[3][:], OP.mult)
    nc.gpsimd.tensor_tensor(ps[4][:], m[1][:], m[3][:], OP.mult)
    nc.vector.tensor_tensor(ps[5][:], m[2][:], m[3][:], OP.mult)

    # per-pair coefficients: t6 cols in PAIRS order, g2m split by readiness
    t6 = st([CS, 6], F32, "t6")
    for i, (s, sp) in enumerate(PAIRS):
        nc.vector.tensor_tensor(t6[:, i:i + 1], rnw_col(s), rnw_col(sp), OP.mult)
    g2mA = st([CS, 3], F32, "g2mA")  # shift-1 pairs (0,1),(1,2),(2,3)
    g2mB0 = st([CS, 1], F32, "g2mB0")  # (0,2)
    g2mB1 = st([CS, 1], F32, "g2mB1")  # (1,3)
    g2mB2 = st([CS, 1], F32, "g2mB2")  # (0,3)
    S1_IDX = [0, 3, 5]  # PAIRS indices of (0,1),(1,2),(2,3)
    S23_IDX = [1, 4, 2]  # (0,2),(1,3),(0,3)
    for j, i in enumerate(S1_IDX):
        nc.vector.scalar_tensor_tensor(
            out=g2mA[:, j:j + 1], in0=gr1[:, j:j + 1], scalar=2.0,
            in1=t6[:, i:i + 1], op0=OP.mult, op1=OP.mult,
        )
    # chain A: ts pre-scales (2x mode, independent of num) + adds
    cpA = [st([CS, B], TIN, f"cpA{j}") for j in range(3)]
    for j, i in enumerate(S1_IDX):
        nc.vector.tensor_scalar_mul(cpA[j][:], ps[i][:], g2mA[:, j:j + 1])
    accA = [st([CS, B], TIN, f"accA{j}") for j in range(3)]
    nc.vector.tensor_tensor(accA[0][:], num[:], cpA[0][:], OP.add)
    nc.vector.tensor_tensor(accA[1][:], cpA[1][:], cpA[2][:], OP.add)

    nc.vector.scalar_tensor_tensor(
        out=g2mB0[:], in0=gr2a_t[:], scalar=2.0,
        in1=t6[:, 1:2], op0=OP.mult, op1=OP.mult,
    )
    nc.vector.scalar_tensor_tensor(
        out=g2mB1[:], in0=gr2b_t[:], scalar=2.0,
        in1=t6[:, 4:5], op0=OP.mult, op1=OP.mult,
    )
    nc.vector.scalar_tensor_tensor(
        out=g2mB2[:], in0=gr3[:, 0:1], scalar=2.0,
        in1=t6[:, 2:3], op0=OP.mult, op1=OP.mult,
    )
    cpB = [st([CS, B], TIN, f"cpB{j}") for j in range(3)]
    nc.vector.tensor_scalar_mul(cpB[0][:], ps[1][:], g2mB0[:])
    nc.vector.tensor_scalar_mul(cpB[1][:], ps[4][:], g2mB1[:])
    nc.vector.tensor_scalar_mul(cpB[2][:], ps[2][:], g2mB2[:])
    accB = [st([CS, B], TIN, f"accB{j}") for j in range(3)]
    nc.vector.tensor_tensor(accB[1][:], cpB[0][:], cpB[1][:], OP.add)

    # ---------- x norms -> rnx broadcast (consumed late) ----------
    ones = st([128, 1], TIN, "ones")
    nc.vector.memset(ones[:], 1.0)
    nx_ps = st([1, B], F32, "nx", psum)
    for k in range(KP):
        mm = nc.tensor.matmul(
            nx_ps[:], ones[:], xsq[:, k, :],
            start=(k == 0), stop=(k == KP - 1),
        )
        if k == 0:
            bass._add_dep_helper(
                mm.ins, last_L_mm.ins, sync=False,
                reason="keep PE on the L matmuls until they finish",
            )
    rnx_row = st([1, B], F32, "rnx_row")
    nc.scalar.activation(rnx_row[:], nx_ps[:], AF.Abs_reciprocal_sqrt)
    ones_row = st([1, 128], F32, "ones_row")
    nc.vector.memset(ones_row[:], 1.0)
    rnx_ps = st([CS, B], F32, "rnx_bc", psum)
    nc.tensor.matmul(rnx_ps[:], ones_row[:, :CS], rnx_row[:], start=True, stop=True)

    # ---------- tail (independent half-chains, per-half out DMA) ---------
    u = st([CS, B], F32, "u")
    nc.vector.tensor_tensor(u[:], num[:], rnx_ps[:], OP.mult)
    H = B // 2
    for h in range(2):
        hs = slice(h * H, (h + 1) * H)
        a2_h = st([CS, H], TIN, f"a2_{h}")
        b2_h = st([CS, H], TIN, f"b2_{h}")
        den_h = st([CS, H], F32, f"den{h}")
        srd_h = st([CS, H], F32, f"srd{h}")
        ot_h = st([CS, H], FP16, f"ot{h}")
        nc.vector.tensor_tensor(a2_h[:], accA[0][:, hs], accA[1][:, hs], OP.add)
        nc.vector.tensor_tensor(b2_h[:], accB[1][:, hs], cpB[2][:, hs], OP.add)
        nc.vector.tensor_tensor(den_h[:], a2_h[:], b2_h[:], OP.add)
        nc.scalar.activation(srd_h[:], den_h[:], AF.Abs_reciprocal_sqrt)
        nc.vector.tensor_tensor(ot_h[:], u[:, hs], srd_h[:], OP.mult)
        nc.sync.dma_start(out_d[:, hs], ot_h[:])


def _build_nc(use_bf16=True, n_iter=1):
    TIN = FP16 if use_bf16 else F32
    nc = bacc.Bacc(
        "TRN2",
        target_bir_lowering=False,
        debug=False,
        enable_asserts=False,
        num_devices=NCORES,
    )
    xT_d = nc.dram_tensor("xT", [D, B], TIN, kind="ExternalInput")
    wT_d = nc.dram_tensor("wT", [128, KP, R], mybir.dt.float8e3, kind="ExternalInput")
    wcm_d = nc.dram_tensor("wcm", [CS, S, D], TIN, kind="ExternalInput")
    out_d = nc.dram_tensor("out", [CS, B], FP16, kind="ExternalOutput")

    with tile.TileContext(nc) as tc:
        with (
            tc.tile_pool(name="main", bufs=1) as pool,
            tc.tile_pool(name="psum", bufs=1, space="PSUM") as psum,
        ):
            for it in range(n_iter):
                _emit_body(nc, pool, psum, xT_d, wT_d, wcm_d, out_d, it, TIN)

    nc.compile()
    return nc


def _get_nc():
    if "nc" not in _CACHED:
        _CACHED["nc"] = _build_nc()
    return _CACHED["nc"]


def _make_in_maps(x, W, use_bf16=True):
    x = np.ascontiguousarray(np.asarray(x, dtype=np.float32))
    W = np.ascontiguousarray(np.asarray(W, dtype=np.float32))
    tin = np.float16 if use_bf16 else np.float32
    xT = np.ascontiguousarray(x.T.astype(tin))  # [D, B]
    in_maps = []
    WSCALE = 64.0  # puts fp8e3 values in normal range; wcm scaled identically
    f8 = ml_dtypes.float8_e3m4
    for i in range(NCORES):
        Wf = W[i * CS:(i + 1) * CS] * WSCALE  # [CS, S, D] f32
        wTf = Wf.transpose(2, 1, 0).reshape(D, R)  # [D, R]
        # pre-pack to the SBUF layout [p, k, r] (d = k*128 + p) so the fp8
        # DMA moves 2000-byte lines (no sub-512B descriptor penalty)
        wT8 = np.ascontiguousarray(
            wTf.reshape(KP, 128, R).transpose(1, 0, 2)).astype(f8)
        wcm = np.ascontiguousarray(Wf.astype(tin))  # [CS, S, D]
        in_maps.append({"xT": xT, "wT": wT8, "wcm": wcm})
    return in_maps


def run(x, W, trace=False):
    nc = _get_nc()
    in_maps = _make_in_maps(x, W)
    res = run_bass_kernel_spmd(
        nc, in_maps, core_ids=list(range(NCORES)), trace=trace
    )
    shards = [res.results[i]["out"].astype(np.float32) for i in range(NCORES)]
    out = np.concatenate([s.T for s in shards], axis=1)  # [B, C]
    return np.ascontiguousarray(out.astype(np.float32)), res


def kernel(x, W):
    out, _ = run(x, W, trace=False)
    return out
